# revision 60
# baseline (speedup 1.0000x reference)
"""Causal multi-head attention block (b=4, s=2048, d=1024, 16 heads) on 8
Trainium2 NeuronCores.

Sharding: tensor-parallel over heads x data-parallel over batch.
Core c handles batch c//2 and head-half c%2 (8 of 16 heads):
  - QKV projection for its 8 heads over all 2048 tokens (bf16 matmuls,
    fp32 PSUM accumulation)
  - causal attention in [k, q] score layout: scores for the even/odd head of
    a pair run concurrently in disjoint PE row-quadrants; softmax denominator
    comes for free from a ones-column appended to the V stationary; the causal
    mask is a precomputed 0/1 multiply on P' (DVE); 1/denom = exp(-ln d) on
    ScalarE, broadcast across partitions with a K=1 matmul
  - unnormalized z^T, per-query reciprocal normalization + V-bias
  - partial O projection over its 512-dim slice (+ b_o/2)
  - pairwise ReduceScatter(add) completes O; each core outputs 1024 tokens.
"""

import sys

import numpy as np
import ml_dtypes

if "/opt/trn_rl_repo" not in sys.path:
    sys.path.insert(0, "/opt/trn_rl_repo")

from contextlib import ExitStack

import concourse.bass as bass
import concourse.tile as tile
from concourse import mybir
import concourse.bass_utils as bass_utils

P = 128
S = 2048          # sequence length
D = 1024          # d_model
DH = 64           # head dim
NHO = 8           # heads per core
DO = 512          # own d-model slice (8 heads * 64)
NW = 1536         # own qkv output cols (512 q + 512 k + 512 v)
FCH = D // P      # 8 feature chunks (contraction over d_model)
NQC = S // 512    # 4 query chunks of 512
dt = mybir.dt
AF = mybir.ActivationFunctionType


def _split_excess_waits(nc):
    """This walrus build allows 1 sync wait per instruction (2 for
    EventSemaphore); Tile's end-of-kernel drain can carry more. Move the
    extras onto preceding NoOps on the same engine."""
    for f in nc.m.functions:
        for bb in f.blocks:
            new_insts = []
            for inst in bb.instructions:
                si = inst.sync_info
                waits = list(si.on_wait) if si and si.on_wait else []
                cap = 2 if isinstance(inst, mybir.InstEventSemaphore) else 1
                if len(waits) > cap:
                    extras, keep = waits[:-cap], waits[-cap:]
                    for i, w in enumerate(extras):
                        new_insts.append(mybir.InstNoOp(
                            name=f"{inst.name}-wsplit{i}", engine=inst.engine,
                            ins=[], outs=[],
                            sync_info=mybir.SyncInfo(on_wait=[w], on_update=[])))
                    si.on_wait = keep
                new_insts.append(inst)
            bb.instructions[:] = new_insts


def _build(use_collective=True, debug=False):
    nc = bass.Bass("TRN2", target_bir_lowering=False, debug=False, num_devices=8)
    xt_d = nc.declare_dram_parameter("xt", [D, S], dt.bfloat16, isOutput=False)
    wqkv_d = nc.declare_dram_parameter("wqkv", [D, NW], dt.bfloat16, isOutput=False)
    wo_d = nc.declare_dram_parameter("wo", [DO, D], dt.bfloat16, isOutput=False)
    bqk_d = nc.declare_dram_parameter("bqk", [P, 8], dt.float32, isOutput=False)
    bv_d = nc.declare_dram_parameter("bv", [P, 4], dt.float32, isOutput=False)
    bo_d = nc.declare_dram_parameter("bo", [1, D], dt.float32, isOutput=False)
    if use_collective:
        out_d = nc.declare_dram_parameter("out", [S // 2, D], dt.bfloat16, isOutput=True)
        opart = nc.dram_tensor("opart", [S, D], dt.bfloat16)
        rsout = nc.dram_tensor("rsout", [S // 2, D], dt.bfloat16)
    else:
        out_d = nc.declare_dram_parameter("out", [S, D], dt.bfloat16, isOutput=True)
        opart = out_d
        rsout = None
    dbg = {}
    if debug:
        for nm in ("dq0", "dk0", "dz0"):
            dbg[nm] = nc.declare_dram_parameter(nm, [P, S], dt.bfloat16, isOutput=True)
        for t in range(4):
            dbg[f"dv{t}"] = nc.declare_dram_parameter(
                f"dv{t}", [P, NHO * (DH + 1)], dt.bfloat16, isOutput=True)
        dbg["ddn0"] = nc.declare_dram_parameter(
            "ddn0", [1, NHO * 512], dt.float32, isOutput=True)
        dbg["drcp0"] = nc.declare_dram_parameter(
            "drcp0", [1, NHO * 512], dt.float32, isOutput=True)
        dbg["dzu0"] = nc.declare_dram_parameter(
            "dzu0", [P, 512], dt.bfloat16, isOutput=True)

    with tile.TileContext(nc) as tc, ExitStack() as ctx:
        const = ctx.enter_context(tc.tile_pool(name="const", bufs=1))
        persist = ctx.enter_context(tc.tile_pool(name="persist", bufs=1))

        # ---- constants -------------------------------------------------
        bqk_sb = const.tile([P, 8], dt.float32, name="bqk", tag="bqk")
        nc.sync.dma_start(out=bqk_sb[:], in_=bqk_d[:])
        bv_sb = const.tile([P, 4], dt.float32, name="bv", tag="bv")
        nc.sync.dma_start(out=bv_sb[:], in_=bv_d[:])
        bo_row = const.tile([1, D], dt.float32, name="bo_row", tag="bo_row")
        nc.sync.dma_start(out=bo_row[:], in_=bo_d[:])
        bo_bc = const.tile([P, D], dt.float32, name="bo_bc", tag="bo_bc")
        ones_col = const.tile([1, P], dt.float32, name="ones_col", tag="ones_col")
        nc.vector.memset(ones_col[:], 1.0)
        ones_col_bf = const.tile([1, P], dt.bfloat16, name="ones_col_bf", tag="ones_col_bf")
        nc.vector.memset(ones_col_bf[:], 1.0)

        # causal P'-mask tiles: mask_i[p, f] = 1 if (f mod 512) - p - 128*i >= 0
        # (both 512-halves identical so one [128,1024] tile serves a full P' tile)
        ones_src = const.tile([P, 1024], dt.bfloat16, name="ones_src", tag="ones_src")
        nc.gpsimd.memset(ones_src[:], 1.0)
        cmask = []
        for i in range(4):
            cm = const.tile([P, 1024], dt.bfloat16, name=f"cmask{i}", tag=f"cmask{i}")
            nc.gpsimd.affine_select(
                cm[:], ones_src[:], pattern=[[0, 2], [1, 512]], base=-128 * i,
                channel_multiplier=-1, compare_op=mybir.AluOpType.is_ge, fill=0.0)
            cmask.append(cm)



        # ---- persistent activations -----------------------------------
        qT = [persist.tile([P, S], dt.bfloat16, name=f"qT{i}", tag=f"qT{i}") for i in range(4)]
        kT = [persist.tile([P, S], dt.bfloat16, name=f"kT{i}", tag=f"kT{i}") for i in range(4)]
        vv = [persist.tile([P, NHO * (DH + 1)], dt.bfloat16, name=f"vv{t}", tag=f"vv{t}")
              for t in range(S // P)]
        z_all = [persist.tile([P, S], dt.bfloat16, name=f"z{i}", tag=f"z{i}") for i in range(4)]
        wo_bf = [persist.tile([P, D], dt.bfloat16, name=f"wo{i}", tag=f"wo{i}") for i in range(4)]

        for dc in range(4):
            nc.sync.dma_start(out=wo_bf[dc][:], in_=wo_d[dc * P:(dc + 1) * P, :])

        # ---- pools (PSUM: shared 2 + scores 4 + z 2 = 8 banks) --------
        ph1 = ctx.enter_context(tc.tile_pool(name="ph1", bufs=1))
        p_pool = ctx.enter_context(tc.tile_pool(name="p_pool", bufs=4))
        dn_pool = ctx.enter_context(tc.tile_pool(name="dn_pool", bufs=2))
        ost_pool = ctx.enter_context(tc.tile_pool(name="ost_pool", bufs=12))
        proj_ps = ctx.enter_context(tc.tile_pool(name="proj_ps", bufs=2, space="PSUM"))
        s_psp = ctx.enter_context(tc.tile_pool(name="s_psp", bufs=2, space="PSUM"))
        zro_psp = ctx.enter_context(tc.tile_pool(name="zro_psp", bufs=2, space="PSUM"))

        dsem = nc.alloc_semaphore("dsem") if use_collective else None
        csem = nc.alloc_semaphore("csem") if use_collective else None
        d2sem = nc.alloc_semaphore("d2sem") if use_collective else None
        n_odma = [0]

        # broadcast b_o/2 to all partitions via a K=1 matmul (one-time)
        for half in range(2):
            bps = proj_ps.tile([P, 512], dt.float32, name="bps", tag="ps")
            nc.tensor.matmul(
                bps[:], lhsT=ones_col[:],
                rhs=bo_row[0:1, half * 512:(half + 1) * 512],
                start=True, stop=True)
            nc.vector.tensor_copy(bo_bc[:, half * 512:(half + 1) * 512], bps[:])

        xt_bf = [ph1.tile([P, S], dt.bfloat16, name=f"xt{f}", tag=f"xt{f}") for f in range(FCH)]
        wq_bf = [ph1.tile([P, NW], dt.bfloat16, name=f"wq{f}", tag=f"wq{f}") for f in range(FCH)]

        def load_w_cols(c0):
            for f in range(FCH):
                nc.sync.dma_start(
                    out=wq_bf[f][:, c0:c0 + 512],
                    in_=wqkv_d[f * P:(f + 1) * P, c0:c0 + 512])

        def load_x_cols(t):
            for f in range(FCH):
                nc.sync.dma_start(
                    out=xt_bf[f][:, t * 512:(t + 1) * 512],
                    in_=xt_d[f * P:(f + 1) * P, t * 512:(t + 1) * 512])

        def kq_proj(base, t, bias_off, dst):
            for n in range(4):
                ps = proj_ps.tile([P, 512], dt.float32, name="ps", tag="ps")
                for f in range(FCH):
                    nc.tensor.matmul(
                        ps[:], lhsT=wq_bf[f][:, base + n * P:base + (n + 1) * P],
                        rhs=xt_bf[f][:, t * 512:(t + 1) * 512],
                        start=(f == 0), stop=(f == FCH - 1))
                nc.vector.tensor_scalar_add(
                    dst[n][:, t * 512:(t + 1) * 512], ps[:],
                    bqk_sb[:, bias_off + n:bias_off + n + 1])

        def v_proj(t16):
            ps = proj_ps.tile([P, 512], dt.float32, name="ps", tag="ps")
            for f in range(FCH):
                nc.tensor.matmul(
                    ps[:], lhsT=xt_bf[f][:, t16 * P:(t16 + 1) * P],
                    rhs=wq_bf[f][:, 1024:1536],
                    start=(f == 0), stop=(f == FCH - 1))
            vview = vv[t16][:].rearrange("p (h c) -> p h c", c=DH + 1)
            nc.vector.tensor_copy(
                vview[:, :, 0:DH], ps[:].rearrange("p (h c) -> p h c", c=DH))
            nc.vector.memset(vview[:, :, DH:DH + 1], 1.0)

        def attention(qc):
            qs = qc * 512
            n_kc = 4 * (qc + 1)
            for ht in range(NHO // 2):
                # heads 2*ht (rows 0:64) and 2*ht+1 (rows 64:128) share the
                # kT/qT tile; their K=64 score matmuls target disjoint PE
                # row-quadrants and run concurrently
                z0 = zro_psp.tile([DH + 1, 512], dt.float32, name="zps0", tag="zro")
                z1 = zro_psp.tile([DH + 1, 512], dt.float32, name="zps1", tag="zro")
                for kc in range(n_kc):
                    di = kc - 4 * qc   # >=0 -> diagonal block
                    s_ps = s_psp.tile([P, 1024], dt.float32, name="sps", tag="sps")
                    nc.tensor.matmul(
                        s_ps[:, 0:512],
                        lhsT=kT[ht][0:DH, kc * P:(kc + 1) * P],
                        rhs=qT[ht][0:DH, qs:qs + 512],
                        start=True, stop=True)
                    nc.tensor.matmul(
                        s_ps[:, 512:1024],
                        lhsT=kT[ht][DH:P, kc * P:(kc + 1) * P],
                        rhs=qT[ht][DH:P, qs:qs + 512],
                        start=True, stop=True)
                    p_t = p_pool.tile([P, 1024], dt.bfloat16, name="pt", tag="pt")
                    nc.scalar.activation(p_t[:], s_ps[:], AF.Exp, scale=0.125)
                    if di >= 0:
                        # causal mask: zero P' where k > q (DVE multiply;
                        # gpsimd is reserved for collective sequencing)
                        nc.vector.tensor_tensor(
                            p_t[:], p_t[:], cmask[di][:], mybir.AluOpType.mult)
                    nc.tensor.matmul(
                        z0[:], lhsT=vv[kc][:, (2 * ht) * 65:(2 * ht) * 65 + 65],
                        rhs=p_t[:, 0:512],
                        start=(kc == 0), stop=(kc == n_kc - 1))
                    nc.tensor.matmul(
                        z1[:], lhsT=vv[kc][:, (2 * ht + 1) * 65:(2 * ht + 1) * 65 + 65],
                        rhs=p_t[:, 512:1024],
                        start=(kc == 0), stop=(kc == n_kc - 1))
                for hp, z_ps in ((0, z0), (DH, z1)):
                    # per-head epilogue, pipelined with later heads.
                    # 1/d = exp(-ln d) on ScalarE (vector.reciprocal is
                    # ~6ns/elem on one partition; this is 2 table lookups).
                    # Both z_ps reads come first so its ring slot frees early.
                    lnrow = dn_pool.tile([1, 512], dt.float32, name="lnrow", tag="lnrow")
                    nc.scalar.activation(lnrow[:], z_ps[DH:DH + 1, :], AF.Ln)
                    zsl = z_all[ht][hp:hp + DH, qs:qs + 512]
                    nc.vector.tensor_copy(zsl, z_ps[0:DH, :])
                    rcprow = dn_pool.tile([1, 512], dt.bfloat16, name="rcprow", tag="rcprow")
                    nc.scalar.activation(rcprow[:], lnrow[:], AF.Exp, scale=-1.0)
                    rbc = zro_psp.tile([P, 512], dt.float32, name="rbc", tag="zro")
                    nc.tensor.matmul(
                        rbc[:], lhsT=ones_col_bf[:], rhs=rcprow[:],
                        start=True, stop=True)
                    nc.vector.tensor_tensor(
                        zsl, zsl, rbc[hp:hp + DH, :], mybir.AluOpType.mult)
                    nc.vector.tensor_scalar_add(
                        zsl, zsl, bv_sb[hp:hp + DH, ht:ht + 1])
            # O projection for this query chunk (own 512-dim slice)
            osts = []
            for t4 in range(4):
                tok = qs + t4 * P
                for no in range(2):
                    ps = zro_psp.tile([P, 512], dt.float32, name="ops", tag="zro")
                    for dc in range(4):
                        nc.tensor.matmul(
                            ps[:], lhsT=z_all[dc][:, tok:tok + P],
                            rhs=wo_bf[dc][:, no * 512:(no + 1) * 512],
                            start=(dc == 0), stop=(dc == 3))
                    ost = ost_pool.tile([P, 512], dt.bfloat16, name="ost", tag="ost")
                    nc.vector.tensor_tensor(
                        ost[:], ps[:], bo_bc[:, no * 512:(no + 1) * 512],
                        mybir.AluOpType.add)
                    osts.append((tok, no, ost))
            if use_collective:
                # DMA this chunk's partials to DRAM, then ReduceScatter
                # 128-token pieces with the pair core while later chunks
                # keep computing.
                for quarter in range(4):
                    with tc.tile_critical():
                        for tok, no, ost in osts[quarter * 2:(quarter + 1) * 2]:
                            nc.gpsimd.dma_start(
                                out=opart[tok:tok + P, no * 512:(no + 1) * 512],
                                in_=ost[:]).then_inc(dsem, 16)
                            n_odma[0] += 1
                        nc.gpsimd.wait_ge(dsem, 16 * n_odma[0])
                        ci = 4 * qc + quarter
                        nc.gpsimd.collective_compute(
                            "ReduceScatter", mybir.AluOpType.add,
                            replica_groups=[[0, 1], [2, 3], [4, 5], [6, 7]],
                            ins=[opart[qs + quarter * P:qs + (quarter + 1) * P, :]],
                            outs=[rsout[ci * DH:(ci + 1) * DH, :]],
                        ).then_inc(csem, 1)
            else:
                for tok, no, ost in osts:
                    nc.sync.dma_start(
                        out=opart[tok:tok + P, no * 512:(no + 1) * 512],
                        in_=ost[:])

        # interleaved emission: attention(qc) only needs token chunks <= qc,
        # so its (ScalarE-heavy) work overlaps the (PE-heavy) projections of
        # later token chunks
        load_w_cols(512)              # K weight columns
        load_x_cols(0)
        kq_proj(512, 0, 4, kT)
        load_w_cols(0)                # Q weight columns
        kq_proj(0, 0, 0, qT)
        load_w_cols(1024)             # V weight columns
        for t16 in range(4):
            v_proj(t16)
        attention(0)
        for t in range(1, 4):
            load_x_cols(t)
            kq_proj(512, t, 4, kT)
            kq_proj(0, t, 0, qT)
            for t16 in range(4 * t, 4 * t + 4):
                v_proj(t16)
            attention(t)

        if debug:
            nc.sync.dma_start(out=dbg["dq0"][:], in_=qT[0][:])
            nc.sync.dma_start(out=dbg["dk0"][:], in_=kT[0][:])
            nc.sync.dma_start(out=dbg["dz0"][:], in_=z_all[0][:])
            for t in range(4):
                nc.sync.dma_start(out=dbg[f"dv{t}"][:], in_=vv[t][:])

        # ---- tail: copy reduced output out ----------------------------
        if use_collective:
            with tc.tile_critical():
                for i in range(4):
                    nc.gpsimd.wait_ge(csem, 4 * (i + 1))
                    nc.gpsimd.dma_start(
                        out=out_d[i * 256:(i + 1) * 256, :],
                        in_=rsout[i * 256:(i + 1) * 256, :]).then_inc(d2sem, 16)
                nc.gpsimd.wait_ge(d2sem, 16 * 4)

    _split_excess_waits(nc)
    return nc


_NC = {}


def _get_nc(use_collective=True):
    if use_collective not in _NC:
        _NC[use_collective] = _build(use_collective)
    return _NC[use_collective]


def _shard(inputs):
    x = np.ascontiguousarray(inputs["x"], dtype=np.float32)
    W_qkv = np.asarray(inputs["W_qkv"], dtype=np.float32)
    b_qkv = np.asarray(inputs["b_qkv"], dtype=np.float32)
    W_o = np.asarray(inputs["W_o"], dtype=np.float32)
    b_o = np.asarray(inputs["b_o"], dtype=np.float32)

    in_maps = []
    for c in range(8):
        b, hh = c // 2, c % 2
        sl = slice(hh * DO, (hh + 1) * DO)
        wq = W_qkv[sl]
        wk = W_qkv[D + hh * DO:D + hh * DO + DO]
        wv = W_qkv[2 * D + hh * DO:2 * D + hh * DO + DO]
        wqkvT = np.ascontiguousarray(np.concatenate([wq, wk, wv], axis=0).T)
        bqk = np.ascontiguousarray(
            np.concatenate([b_qkv[hh * DO:hh * DO + DO],
                            b_qkv[D + hh * DO:D + hh * DO + DO]])
            .reshape(8, P).T)
        bv = np.ascontiguousarray(
            b_qkv[2 * D + hh * DO:2 * D + hh * DO + DO].reshape(4, P).T)
        woT = np.ascontiguousarray(W_o.T[sl])
        in_maps.append({
            "xt": np.ascontiguousarray(x[b].T).astype(ml_dtypes.bfloat16),
            "wqkv": wqkvT.astype(ml_dtypes.bfloat16),
            "wo": woT.astype(ml_dtypes.bfloat16),
            "bqk": bqk,
            "bv": bv,
            "bo": np.ascontiguousarray((0.5 * b_o).reshape(1, D)),
        })
    return in_maps


def _unshard(results, batch, use_collective=True):
    out = np.empty((batch, S, D), dtype=np.float32)
    for b in range(batch):
        if use_collective:
            # 256-token ReduceScatter pieces: piece ci covers tokens
            # [ci*256, (ci+1)*256); rank r of the pair holds its r-th 128 rows
            # at rsout rows [ci*128, (ci+1)*128)
            for ci in range(16):
                out[b, ci * 128:ci * 128 + 64] = \
                    results[2 * b]["out"][ci * 64:(ci + 1) * 64].astype(np.float32)
                out[b, ci * 128 + 64:(ci + 1) * 128] = \
                    results[2 * b + 1]["out"][ci * 64:(ci + 1) * 64].astype(np.float32)
        else:
            out[b] = (results[2 * b]["out"].astype(np.float32)
                      + results[2 * b + 1]["out"].astype(np.float32))
    return out


def _run(inputs, trace=False, trace_kwargs=None, use_collective=True):
    nc = _get_nc(use_collective)
    in_maps = _shard(inputs)
    if trace:
        import types
        if "antenv.axon_hooks" not in sys.modules:
            mod = types.ModuleType("antenv.axon_hooks")
            _hook = [None]
            mod.set_axon_ntff_profile_hook = lambda h: _hook.__setitem__(0, h)
            mod.get_axon_ntff_profile_hook = lambda: _hook[0]
            sys.modules["antenv.axon_hooks"] = mod
            from trn_agent_boot.trn_boot import _ntff_profile_via_ctypes
            mod.set_axon_ntff_profile_hook(
                _ntff_profile_via_ctypes("/opt/axon/libaxon_pjrt.so"))
        bass_utils.upload_artifacts = lambda tmpdir: tmpdir
    res = bass_utils.run_bass_kernel_spmd(
        nc, in_maps, core_ids=list(range(8)), trace=trace,
        **(trace_kwargs or {}))
    out = _unshard(res.results, inputs["x"].shape[0], use_collective)
    return out, res


def kernel(**inputs) -> np.ndarray:
    out, _ = _run(inputs, trace=False)
    return out
